# revision 16
# baseline (speedup 1.0000x reference)
"""Trainium2 Bass kernel: BigramHashEmbedding (hash -> embed gather -> proj -> scale).

Computation (per batch row, one NeuronCore per row, 8 rows total):
    h[0]  = 10239
    h[j]  = (36313*t[j] ^ 27191*t[j-1]) % 10239          (int32, j >= 1)
    e     = embed_weight[h]                               [S, 128] gather
    out   = (e @ proj_weight.T) * scale                   [S, 512]

Device strategy per core (S = 8192 tokens):
  * tokens are viewed int32 (lo-words of int64 if needed) and loaded into
    SBUF in [16, 512] layout (partition p holds tokens 512p..512p+511),
    replicated 8x across the 128 partitions (the dma_gather index tile must
    be "wrapped in 16 partitions and replicated across cores").
  * the bigram hash is computed on DVE/ACT with fp32-exact arithmetic:
    products are split (36313 = 141*256 + 217, 27191 = 106*256 + 55) so that
    every arithmetic op stays below 2^24 (the vector ALU is fp32 internally);
    >=2^24 values only ever pass through bitwise ops (shift/and/xor), which
    are bit-exact.  The mod-10239 is a limb decomposition
    X = u*2^21 + v*2^8 + w  ->  y = u*8396 + (v<<8) + w  (y < 2^24)
    followed by one fp32 reciprocal-multiply quotient and two +-m fixups.
  * one dma_gather per 2048 tokens fetches embedding rows straight from the
    DRAM table into SBUF slot layout [128, 16, 128] (row k%128 / block k//128).
    Gather slot k holds token 512*(k%16) + k//16.
  * per 128-token block: PE transpose (identity) -> PSUM -> SBUF, then
    PE matmul eT.T @ projT_scaled -> PSUM [128, 512] -> SBUF -> HWDGE DMA to
    the matching strided rows of the output.  Matmuls run as float32r
    (single-pass fp32, 1 cycle/row instead of 4).
  * proj [512, 128] is transposed on the PE at setup into projT [128, 512]
    and pre-scaled by `scale` (broadcast via a K=1 matmul with a ones row).
"""

from contextlib import ExitStack

import numpy as np

import concourse.bacc as bacc
import concourse.bass as bass
import concourse.mybir as mybir
import concourse.tile as tile
from concourse.bass_utils import run_bass_kernel_spmd
from concourse.masks import make_identity

AL = mybir.AluOpType
F32 = mybir.dt.float32
F32R = mybir.dt.float32r
I32 = mybir.dt.int32
I16 = mybir.dt.int16

B = 8           # batch rows == cores
S = 8192        # tokens per core
V = 10240       # hash table rows
D = 128         # embed dim
M = 512         # model dim
P = 128
MOD = 10239     # hash modulus (HASH_SIZE - 1)
SPT = S // 16   # tokens per index-partition = 512
NG = 8          # sub-gathers
TPG = S // NG   # tokens per gather = 2048
CPG = SPT // NG  # idx columns per gather group = 128
NB = S // P     # 128-token blocks = 64
BPG = NB // NG  # blocks per gather = 16
HASH_CHUNKS = (64, 64, 128, 256)   # progressive: short first chain, wide later
assert sum(HASH_CHUNKS) == SPT

# 36313 = 141*256 + 217 ; 27191 = 106*256 + 55
A_HI, A_LO = 141, 217
B_HI, B_LO = 106, 55
C21 = 8396      # 2^21 mod 10239
INV_M = 1.0 / MOD

USE_ACT_MUL = True   # run the big multiplies on the Scalar (ACT) engine
N_QUEUES = 4         # SWDGE queues for the gathers
USE_F32R = True      # float32r single-pass matmul (1 cyc/row instead of 4)
SIM_COMPAT = False   # add the >=MOD fixup (needed only under CoreSim's trunc convert)


def _mul(nc, out, in_, const):
    if USE_ACT_MUL:
        nc.scalar.mul(out, in_, float(const))
    else:
        nc.vector.tensor_scalar_mul(out, in_, float(const))


def _hash_chunk(nc, tmp, idx, toks_v, tm1, mask, offs, cs, n):
    """Emit ops computing idx[:, cs:cs+n] (int16 hash values).

    toks_v: [128, SPT, W] int32 view of the token tile (lo word at w=0).
    tm1:    [128, 1] int32, t[512p - 1] per partition (garbage at p%16==0).
    mask:   [128, 1] int32, (p % 16) != 0.
    offs:   [128, 1] int32, 10239 * (p % 16 == 0).
    """
    g = 0 if cs == 0 else 1  # only the cs==0 chunk handles the row head

    tcur = toks_v[:, cs:cs + n, 0:1]
    p1 = tmp.tile([P, n], I32, tag=f"p1_{n}")
    p2 = tmp.tile([P, n], I32, tag=f"p2_{n}")
    q1 = tmp.tile([P, n], I32, tag=f"q1_{n}")
    q2 = tmp.tile([P, n], I32, tag=f"q2_{n}")
    _mul(nc, p1[:], tcur, A_LO)
    _mul(nc, p2[:], tcur, A_HI)
    if g == 0:
        tprev = toks_v[:, 0:n - 1, 0:1]
        _mul(nc, q1[:, 1:n], tprev, B_LO)
        _mul(nc, q2[:, 1:n], tprev, B_HI)
        _mul(nc, q1[:, 0:1], tm1[:], B_LO)
        _mul(nc, q2[:, 0:1], tm1[:], B_HI)
    else:
        tprev = toks_v[:, cs - 1:cs + n - 1, 0:1]
        _mul(nc, q1[:], tprev, B_LO)
        _mul(nc, q2[:], tprev, B_HI)

    # A>>8 = p2 + (p1>>8);  B>>8 = q2 + (q1>>8)   (both < 2^23, exact)
    # (the compiler rejects bitwise op0 fused with arith op1, so shift and
    # add are separate instructions)
    ah = tmp.tile([P, n], I32, tag=f"ah_{n}")
    bh = tmp.tile([P, n], I32, tag=f"bh_{n}")
    t1 = tmp.tile([P, n], I32, tag=f"t1_{n}")
    nc.vector.tensor_single_scalar(t1[:], p1[:], 8, op=AL.logical_shift_right)
    nc.vector.tensor_add(ah[:], t1[:], p2[:])
    nc.vector.tensor_single_scalar(t1[:], q1[:], 8, op=AL.logical_shift_right)
    nc.vector.tensor_add(bh[:], t1[:], q2[:])
    # X>>8 and X low byte (in low 8 bits of xl)
    xh = tmp.tile([P, n], I32, tag=f"xh_{n}")
    xl = tmp.tile([P, n], I32, tag=f"xl_{n}")
    nc.vector.tensor_tensor(xh[:], ah[:], bh[:], op=AL.bitwise_xor)
    nc.vector.tensor_tensor(xl[:], p1[:], q1[:], op=AL.bitwise_xor)

    # y = (xh>>13)*8396 + ((xh & 8191) << 8) + (xl & 255)   ( < 2^24 )
    w1 = tmp.tile([P, n], I32, tag=f"w1_{n}")
    w2 = tmp.tile([P, n], I32, tag=f"w2_{n}")
    nc.vector.tensor_single_scalar(w1[:], xh[:], 13, op=AL.logical_shift_right)
    nc.vector.tensor_scalar_mul(w1[:], w1[:], float(C21))
    nc.vector.tensor_scalar(w2[:], xh[:], 8191, 8,
                            op0=AL.bitwise_and, op1=AL.logical_shift_left)
    w3 = tmp.tile([P, n], I32, tag=f"w3_{n}")
    nc.vector.tensor_add(w3[:], w1[:], w2[:])
    y = tmp.tile([P, n], I32, tag=f"y_{n}")
    nc.vector.tensor_single_scalar(y[:], xl[:], 255, op=AL.bitwise_and)
    nc.vector.tensor_add(y[:], y[:], w3[:])

    # r = y - trunc(y/m)*m, then two +-m fixups
    qt = tmp.tile([P, n], I32, tag=f"qt_{n}")
    _mul(nc, qt[:], y[:], INV_M)
    r = tmp.tile([P, n], I32, tag=f"r_{n}")
    nc.vector.scalar_tensor_tensor(r[:], qt[:], -float(MOD), y[:],
                                   op0=AL.mult, op1=AL.add)
    if SIM_COMPAT:
        f1 = tmp.tile([P, n], I32, tag=f"f1_{n}")
        nc.vector.tensor_single_scalar(f1[:], r[:], float(MOD), op=AL.is_ge)
        nc.vector.scalar_tensor_tensor(r[:], f1[:], -float(MOD), r[:],
                                       op0=AL.mult, op1=AL.add)
    f2 = tmp.tile([P, n], I32, tag=f"f2_{n}")
    nc.vector.tensor_single_scalar(f2[:], r[:], 0.0, op=AL.is_lt)
    nc.vector.scalar_tensor_tensor(r[:], f2[:], float(MOD), r[:],
                                   op0=AL.mult, op1=AL.add)

    if g == 0:
        # token 0 (partition p%16==0, col 0): h = MOD
        nc.vector.tensor_mul(r[:, 0:1], r[:, 0:1], mask[:])
        nc.vector.tensor_add(r[:, 0:1], r[:, 0:1], offs[:])

    nc.vector.tensor_copy(idx[:, cs:cs + n], r[:])


def body(ctx: ExitStack, tc: tile.TileContext, out_ap, tok_ap, table_ap,
         proj_ap, scale_ap, W: int):
    """Emit the per-core kernel. tok_ap is int32 [S*W] (W=2 -> int64 lo/hi)."""
    nc = tc.nc

    const = ctx.enter_context(tc.tile_pool(name="const", bufs=1))
    tmp = ctx.enter_context(tc.tile_pool(name="tmp", bufs=2))
    gpool = ctx.enter_context(tc.tile_pool(name="gpool", bufs=1))
    et_pool = ctx.enter_context(tc.tile_pool(name="et", bufs=6))
    o_pool = ctx.enter_context(tc.tile_pool(name="osb", bufs=8))

    # ---- tokens first (they gate the hash -> gather critical path) ----
    FW = SPT * W
    tokv = tok_ap.rearrange("(p f) -> p f", p=16)
    toks = const.tile([P, FW], I32)
    tm1 = const.tile([P, W], I32)
    nc.gpsimd.memset(tm1[:], 0)
    nc.sync.dma_start(toks[:], tokv[None].broadcast_to([8, 16, FW]))
    for r in range(8):
        # t[512q - 1] for q>=1: last element of the previous partition
        nc.sync.dma_start(tm1[16 * r + 1:16 * (r + 1), :],
                          tokv[0:15, FW - W:FW])
    toks_v = toks.rearrange("p (s w) -> p s w", w=W)

    # partition masks for the token-0 override
    pi = const.tile([P, 1], I32)
    nc.gpsimd.iota(pi[:], pattern=[[0, 1]], base=0, channel_multiplier=1)
    mask = const.tile([P, 1], I32)
    nc.vector.tensor_single_scalar(mask[:], pi[:], 15, op=AL.bitwise_and)
    nc.vector.tensor_single_scalar(mask[:], mask[:], 0.0, op=AL.not_equal)
    offs = const.tile([P, 1], I32)
    nc.vector.tensor_scalar(offs[:], mask[:], -float(MOD), float(MOD),
                            op0=AL.mult, op1=AL.add)

    idx = const.tile([P, SPT], I16)
    g_sb = gpool.tile([P, NB, P], F32R if USE_F32R else F32)

    # hash + gathers (each chunk covers whole gathers; gather = CPG columns)
    cs = 0
    for n in HASH_CHUNKS:
        _hash_chunk(nc, tmp, idx, toks_v, tm1[:, 0:1], mask, offs, cs, n)
        for g in range(cs // CPG, (cs + n) // CPG):
            nc.gpsimd.dma_gather(
                g_sb[:, BPG * g:BPG * (g + 1), :],
                table_ap.bitcast(F32R) if USE_F32R else table_ap,
                idx[:, CPG * g:CPG * (g + 1)],
                num_idxs=TPG,
                num_idxs_reg=TPG,
                elem_size=D,
                single_packet=False,
                queue_num=g % N_QUEUES,
            )
        cs += n

    # ---- setup: identity, projT (transposed + pre-scaled) ----
    ps_setup = tc.alloc_tile_pool(name="ps_setup", bufs=1, space="PSUM")
    ident_f = const.tile([P, P], F32)
    make_identity(nc, ident_f[:])
    if USE_F32R:
        # f32r consumers need f32r producers; DVE copy does the rounding
        ident = const.tile([P, P], F32R)
        nc.vector.tensor_copy(ident[:], ident_f[:])
    else:
        ident = ident_f

    # scale broadcast [1,1] -> [128,1] via K=1 matmul with a ones row
    sc_in = const.tile([1, 1], F32)
    nc.sync.dma_start(sc_in[:], scale_ap)
    ones = const.tile([1, P], F32)
    nc.gpsimd.memset(ones[:], 1.0)
    ps_sc = ps_setup.tile([P, 1], F32, space="PSUM", tag="ps_sc")
    nc.tensor.matmul(ps_sc[:], lhsT=ones[:], rhs=sc_in[:], start=True, stop=True)
    sc_b = const.tile([P, 1], F32)
    nc.vector.tensor_copy(sc_b[:], ps_sc[:])

    projT = const.tile([P, M], F32R if USE_F32R else F32)
    for c in range(M // P):
        pch = tmp.tile([P, P], F32, tag="pch")
        nc.sync.dma_start(pch[:], proj_ap[c * P:(c + 1) * P, :])
        ps_t = ps_setup.tile([P, P], F32, space="PSUM", tag="ps_t")
        nc.tensor.transpose(ps_t[:], pch[:], ident_f[:])
        nc.vector.tensor_copy(projT[:, c * P:(c + 1) * P], ps_t[:])
    nc.vector.tensor_scalar_mul(projT[:], projT[:], sc_b[:, 0:1])
    ps_setup.release()
    ps_small = ctx.enter_context(tc.tile_pool(name="ps_small", bufs=3, space="PSUM"))
    ps_big = ctx.enter_context(tc.tile_pool(name="ps_big", bufs=4, space="PSUM"))

    # DRAM-side dims iterate (r outer, q inner, m) so the flat order pairs
    # SBUF partition p = r*16 + q with output row 512q + 8s + r.
    out_r = out_ap.rearrange("(q s r) m -> r q s m", q=16, s=NB, r=8)

    for b in range(NB):
        ps_et = ps_small.tile([P, P], F32R if USE_F32R else F32, space="PSUM", tag="ps_et")
        nc.tensor.transpose(ps_et[:], g_sb[:, b, :], ident[:])
        et = et_pool.tile([P, P], F32R if USE_F32R else F32, tag="et")
        if b % 2 == 0:
            nc.vector.tensor_copy(et[:], ps_et[:])
        else:
            nc.scalar.copy(et[:], ps_et[:])
        ps_o = ps_big.tile([P, M], F32, space="PSUM", tag="ps_o")
        nc.tensor.matmul(ps_o[:], lhsT=et[:], rhs=projT[:],
                         start=True, stop=True)
        o_sb = o_pool.tile([P, M], F32, tag="o_sb")
        if b % 2 == 0:
            nc.scalar.copy(o_sb[:], ps_o[:])
        else:
            nc.vector.tensor_copy(o_sb[:], ps_o[:])
        nc.sync.dma_start(out_r[:, :, b], o_sb[:])


_CACHE: dict = {}


def _build(W: int):
    if W in _CACHE:
        return _CACHE[W]
    nc = bacc.Bacc("TRN2", target_bir_lowering=False, debug=False,
                   num_swdge_queues=N_QUEUES, dynamic_dma_scratch_size=65536)
    tok = nc.dram_tensor("token_ids", [S * W], I32, kind="ExternalInput").ap()
    table = nc.dram_tensor("embed_weight", [V, D], F32, kind="ExternalInput").ap()
    proj = nc.dram_tensor("proj_weight", [M, D], F32, kind="ExternalInput").ap()
    scale = nc.dram_tensor("scale", [1, 1], F32, kind="ExternalInput").ap()
    out = nc.dram_tensor("out", [S, M], F32, kind="ExternalOutput").ap()
    with tile.TileContext(nc) as tc:
        with ExitStack() as ctx:
            body(ctx, tc, out, tok, table, proj, scale, W)
    nc.compile()
    _CACHE[W] = nc
    return nc


def kernel(token_ids: np.ndarray, embed_weight: np.ndarray,
           proj_weight: np.ndarray, scale: np.ndarray) -> np.ndarray:
    token_ids = np.ascontiguousarray(token_ids)
    assert token_ids.shape == (B, S), token_ids.shape
    W = 2 if token_ids.dtype.itemsize == 8 else 1
    tok32 = token_ids.view(np.int32).reshape(B, S * W)
    table = np.ascontiguousarray(embed_weight, dtype=np.float32)
    proj = np.ascontiguousarray(proj_weight, dtype=np.float32)
    sc = np.asarray(scale, dtype=np.float32).reshape(1, 1)

    nc = _build(W)
    in_maps = [
        {
            "token_ids": np.ascontiguousarray(tok32[i]),
            "embed_weight": table,
            "proj_weight": proj,
            "scale": sc,
        }
        for i in range(B)
    ]
    res = run_bass_kernel_spmd(nc, in_maps, core_ids=list(range(B)))
    return np.stack([r["out"] for r in res.results], axis=0)


# revision 17
# speedup vs baseline: 1.0115x; 1.0115x over previous
"""Trainium2 Bass kernel: BigramHashEmbedding (hash -> embed gather -> proj -> scale).

Computation (per batch row, one NeuronCore per row, 8 rows total):
    h[0]  = 10239
    h[j]  = (36313*t[j] ^ 27191*t[j-1]) % 10239          (int32, j >= 1)
    e     = embed_weight[h]                               [S, 128] gather
    out   = (e @ proj_weight.T) * scale                   [S, 512]

Device strategy per core (S = 8192 tokens):
  * tokens are viewed int32 (lo-words of int64 if needed) and loaded into
    SBUF in [16, 512] layout (partition p holds tokens 512p..512p+511),
    replicated 8x across the 128 partitions (the dma_gather index tile must
    be "wrapped in 16 partitions and replicated across cores").
  * the bigram hash is computed on DVE/ACT with fp32-exact arithmetic:
    products are split (36313 = 141*256 + 217, 27191 = 106*256 + 55) so that
    every arithmetic op stays below 2^24 (the vector ALU is fp32 internally);
    >=2^24 values only ever pass through bitwise ops (shift/and/xor), which
    are bit-exact.  The mod-10239 is a limb decomposition
    X = u*2^21 + v*2^8 + w  ->  y = u*8396 + (v<<8) + w  (y < 2^24)
    followed by one fp32 reciprocal-multiply quotient and two +-m fixups.
  * one dma_gather per 2048 tokens fetches embedding rows straight from the
    DRAM table into SBUF slot layout [128, 16, 128] (row k%128 / block k//128).
    Gather slot k holds token 512*(k%16) + k//16.
  * per 128-token block: PE transpose (identity) -> PSUM -> SBUF, then
    PE matmul eT.T @ projT_scaled -> PSUM [128, 512] -> SBUF -> HWDGE DMA to
    the matching strided rows of the output.  Matmuls run as float32r
    (single-pass fp32, 1 cycle/row instead of 4).
  * proj [512, 128] is transposed on the PE at setup into projT [128, 512]
    and pre-scaled by `scale` (broadcast via a K=1 matmul with a ones row).
"""

from contextlib import ExitStack

import numpy as np

import concourse.bacc as bacc
import concourse.bass as bass
import concourse.mybir as mybir
import concourse.tile as tile
from concourse.bass_utils import run_bass_kernel_spmd
from concourse.masks import make_identity

AL = mybir.AluOpType
F32 = mybir.dt.float32
F32R = mybir.dt.float32r
I32 = mybir.dt.int32
I16 = mybir.dt.int16

B = 8           # batch rows == cores
S = 8192        # tokens per core
V = 10240       # hash table rows
D = 128         # embed dim
M = 512         # model dim
P = 128
MOD = 10239     # hash modulus (HASH_SIZE - 1)
SPT = S // 16   # tokens per index-partition = 512
NG = 8          # sub-gathers
TPG = S // NG   # tokens per gather = 2048
CPG = SPT // NG  # idx columns per gather group = 128
NB = S // P     # 128-token blocks = 64
BPG = NB // NG  # blocks per gather = 16
HASH_CHUNKS = (64, 64, 128, 256)   # progressive: short first chain, wide later
assert sum(HASH_CHUNKS) == SPT

# 36313 = 141*256 + 217 ; 27191 = 106*256 + 55
A_HI, A_LO = 141, 217
B_HI, B_LO = 106, 55
C21 = 8396      # 2^21 mod 10239
INV_M = 1.0 / MOD

USE_ACT_MUL = True   # run the big multiplies on the Scalar (ACT) engine
N_QUEUES = 4         # SWDGE queues for the gathers
USE_F32R = True      # float32r single-pass matmul (1 cyc/row instead of 4)
SIM_COMPAT = False   # add the >=MOD fixup (needed only under CoreSim's trunc convert)


def _mul(nc, out, in_, const):
    if USE_ACT_MUL:
        nc.scalar.mul(out, in_, float(const))
    else:
        nc.vector.tensor_scalar_mul(out, in_, float(const))


def _hash_chunk(nc, tmp, idx, toks_v, tm1, mask, offs, cs, n):
    """Emit ops computing idx[:, cs:cs+n] (int16 hash values).

    toks_v: [128, SPT, W] int32 view of the token tile (lo word at w=0).
    tm1:    [128, 1] int32, t[512p - 1] per partition (garbage at p%16==0).
    mask:   [128, 1] int32, (p % 16) != 0.
    offs:   [128, 1] int32, 10239 * (p % 16 == 0).
    """
    g = 0 if cs == 0 else 1  # only the cs==0 chunk handles the row head

    tcur = toks_v[:, cs:cs + n, 0:1]
    p1 = tmp.tile([P, n], I32, tag=f"p1_{n}")
    p2 = tmp.tile([P, n], I32, tag=f"p2_{n}")
    q1 = tmp.tile([P, n], I32, tag=f"q1_{n}")
    q2 = tmp.tile([P, n], I32, tag=f"q2_{n}")
    _mul(nc, p1[:], tcur, A_LO)
    _mul(nc, p2[:], tcur, A_HI)
    if g == 0:
        tprev = toks_v[:, 0:n - 1, 0:1]
        _mul(nc, q1[:, 1:n], tprev, B_LO)
        _mul(nc, q2[:, 1:n], tprev, B_HI)
        _mul(nc, q1[:, 0:1], tm1[:], B_LO)
        _mul(nc, q2[:, 0:1], tm1[:], B_HI)
    else:
        tprev = toks_v[:, cs - 1:cs + n - 1, 0:1]
        _mul(nc, q1[:], tprev, B_LO)
        _mul(nc, q2[:], tprev, B_HI)

    # A>>8 = p2 + (p1>>8);  B>>8 = q2 + (q1>>8)   (both < 2^23, exact)
    # (the compiler rejects bitwise op0 fused with arith op1, so shift and
    # add are separate instructions)
    ah = tmp.tile([P, n], I32, tag=f"ah_{n}")
    bh = tmp.tile([P, n], I32, tag=f"bh_{n}")
    t1 = tmp.tile([P, n], I32, tag=f"t1_{n}")
    nc.vector.tensor_single_scalar(t1[:], p1[:], 8, op=AL.logical_shift_right)
    nc.vector.tensor_add(ah[:], t1[:], p2[:])
    nc.vector.tensor_single_scalar(t1[:], q1[:], 8, op=AL.logical_shift_right)
    nc.vector.tensor_add(bh[:], t1[:], q2[:])
    # X>>8 and X low byte (in low 8 bits of xl)
    xh = tmp.tile([P, n], I32, tag=f"xh_{n}")
    xl = tmp.tile([P, n], I32, tag=f"xl_{n}")
    nc.vector.tensor_tensor(xh[:], ah[:], bh[:], op=AL.bitwise_xor)
    nc.vector.tensor_tensor(xl[:], p1[:], q1[:], op=AL.bitwise_xor)

    # y = (xh>>13)*8396 + ((xh & 8191) << 8) + (xl & 255)   ( < 2^24 )
    w1 = tmp.tile([P, n], I32, tag=f"w1_{n}")
    w2 = tmp.tile([P, n], I32, tag=f"w2_{n}")
    nc.vector.tensor_single_scalar(w1[:], xh[:], 13, op=AL.logical_shift_right)
    nc.vector.tensor_scalar_mul(w1[:], w1[:], float(C21))
    nc.vector.tensor_scalar(w2[:], xh[:], 8191, 8,
                            op0=AL.bitwise_and, op1=AL.logical_shift_left)
    w3 = tmp.tile([P, n], I32, tag=f"w3_{n}")
    nc.vector.tensor_add(w3[:], w1[:], w2[:])
    y = tmp.tile([P, n], I32, tag=f"y_{n}")
    nc.vector.tensor_single_scalar(y[:], xl[:], 255, op=AL.bitwise_and)
    nc.vector.tensor_add(y[:], y[:], w3[:])

    # r = y - trunc(y/m)*m, then two +-m fixups
    qt = tmp.tile([P, n], I32, tag=f"qt_{n}")
    _mul(nc, qt[:], y[:], INV_M)
    r = tmp.tile([P, n], I32, tag=f"r_{n}")
    nc.vector.scalar_tensor_tensor(r[:], qt[:], -float(MOD), y[:],
                                   op0=AL.mult, op1=AL.add)
    if SIM_COMPAT:
        f1 = tmp.tile([P, n], I32, tag=f"f1_{n}")
        nc.vector.tensor_single_scalar(f1[:], r[:], float(MOD), op=AL.is_ge)
        nc.vector.scalar_tensor_tensor(r[:], f1[:], -float(MOD), r[:],
                                       op0=AL.mult, op1=AL.add)
    f2 = tmp.tile([P, n], I32, tag=f"f2_{n}")
    nc.vector.tensor_single_scalar(f2[:], r[:], 0.0, op=AL.is_lt)
    nc.vector.scalar_tensor_tensor(r[:], f2[:], float(MOD), r[:],
                                   op0=AL.mult, op1=AL.add)

    if g == 0:
        # token 0 (partition p%16==0, col 0): h = MOD
        nc.vector.tensor_mul(r[:, 0:1], r[:, 0:1], mask[:])
        nc.vector.tensor_add(r[:, 0:1], r[:, 0:1], offs[:])

    nc.vector.tensor_copy(idx[:, cs:cs + n], r[:])


def body(ctx: ExitStack, tc: tile.TileContext, out_ap, tok_ap, table_ap,
         proj_ap, scale_ap, W: int):
    """Emit the per-core kernel. tok_ap is int32 [S*W] (W=2 -> int64 lo/hi)."""
    nc = tc.nc

    const = ctx.enter_context(tc.tile_pool(name="const", bufs=1))
    tmp = ctx.enter_context(tc.tile_pool(name="tmp", bufs=2))
    gpool = ctx.enter_context(tc.tile_pool(name="gpool", bufs=1))
    et_pool = ctx.enter_context(tc.tile_pool(name="et", bufs=6))
    o_pool = ctx.enter_context(tc.tile_pool(name="osb", bufs=8))

    # ---- tokens first (they gate the hash -> gather critical path) ----
    FW = SPT * W
    tokv = tok_ap.rearrange("(p f) -> p f", p=16)
    toks = const.tile([P, FW], I32)
    tm1 = const.tile([P, W], I32)
    nc.gpsimd.memset(tm1[:], 0)
    nc.sync.dma_start(toks[:], tokv[None].broadcast_to([8, 16, FW]))
    for r in range(8):
        # t[512q - 1] for q>=1: last element of the previous partition
        nc.sync.dma_start(tm1[16 * r + 1:16 * (r + 1), :],
                          tokv[0:15, FW - W:FW])
    toks_v = toks.rearrange("p (s w) -> p s w", w=W)

    # partition masks for the token-0 override
    pi = const.tile([P, 1], I32)
    nc.gpsimd.iota(pi[:], pattern=[[0, 1]], base=0, channel_multiplier=1)
    mask = const.tile([P, 1], I32)
    nc.vector.tensor_single_scalar(mask[:], pi[:], 15, op=AL.bitwise_and)
    nc.vector.tensor_single_scalar(mask[:], mask[:], 0.0, op=AL.not_equal)
    offs = const.tile([P, 1], I32)
    nc.vector.tensor_scalar(offs[:], mask[:], -float(MOD), float(MOD),
                            op0=AL.mult, op1=AL.add)

    idx = const.tile([P, SPT], I16)
    g_sb = gpool.tile([P, NB, P], F32R if USE_F32R else F32)

    # hash + gathers (each chunk covers whole gathers; gather = CPG columns)
    cs = 0
    for n in HASH_CHUNKS:
        _hash_chunk(nc, tmp, idx, toks_v, tm1[:, 0:1], mask, offs, cs, n)
        for g in range(cs // CPG, (cs + n) // CPG):
            nc.gpsimd.dma_gather(
                g_sb[:, BPG * g:BPG * (g + 1), :],
                table_ap.bitcast(F32R) if USE_F32R else table_ap,
                idx[:, CPG * g:CPG * (g + 1)],
                num_idxs=TPG,
                num_idxs_reg=TPG,
                elem_size=D,
                single_packet=False,
                queue_num=g % N_QUEUES,
            )
        cs += n

    # ---- setup: identity, projT (transposed + pre-scaled) ----
    ps_setup = tc.alloc_tile_pool(name="ps_setup", bufs=1, space="PSUM")
    ident_f = const.tile([P, P], F32)
    make_identity(nc, ident_f[:])
    if USE_F32R:
        # f32r consumers need f32r producers; DVE copy does the rounding
        ident = const.tile([P, P], F32R)
        nc.vector.tensor_copy(ident[:], ident_f[:])
    else:
        ident = ident_f

    # scale broadcast [1,1] -> [128,1] via K=1 matmul with a ones row
    sc_in = const.tile([1, 1], F32)
    nc.sync.dma_start(sc_in[:], scale_ap)
    ones = const.tile([1, P], F32)
    nc.gpsimd.memset(ones[:], 1.0)
    ps_sc = ps_setup.tile([P, 1], F32, space="PSUM", tag="ps_sc")
    nc.tensor.matmul(ps_sc[:], lhsT=ones[:], rhs=sc_in[:], start=True, stop=True)
    sc_b = const.tile([P, 1], F32)
    nc.vector.tensor_copy(sc_b[:], ps_sc[:])

    projT = const.tile([P, M], F32R if USE_F32R else F32)
    for c in range(M // P):
        pch = tmp.tile([P, P], F32, tag="pch")
        nc.sync.dma_start(pch[:], proj_ap[c * P:(c + 1) * P, :])
        ps_t = ps_setup.tile([P, P], F32, space="PSUM", tag="ps_t")
        nc.tensor.transpose(ps_t[:], pch[:], ident_f[:])
        nc.vector.tensor_copy(projT[:, c * P:(c + 1) * P], ps_t[:])
    nc.vector.tensor_scalar_mul(projT[:], projT[:], sc_b[:, 0:1])
    ps_setup.release()
    ps_small = ctx.enter_context(tc.tile_pool(name="ps_small", bufs=4, space="PSUM"))
    ps_big = ctx.enter_context(tc.tile_pool(name="ps_big", bufs=4, space="PSUM"))

    # DRAM-side dims iterate (r outer, q inner, m) so the flat order pairs
    # SBUF partition p = r*16 + q with output row 512q + 8s + r.
    out_r = out_ap.rearrange("(q s r) m -> r q s m", q=16, s=NB, r=8)

    # Software-pipelined block loop: the PE issues transpose(b+LAG) before
    # matmul(b) so the PSUM->SBUF copy latency of eT stays off the PE's
    # in-order critical path.
    LAG = 2
    ets = {}

    def emit_trans(b):
        ps_et = ps_small.tile([P, P], F32R if USE_F32R else F32, space="PSUM",
                              tag="ps_et", name=f"ps_et{b}")
        nc.tensor.transpose(ps_et[:], g_sb[:, b, :], ident[:])
        et = et_pool.tile([P, P], F32R if USE_F32R else F32, tag="et",
                          name=f"et{b}")
        if b % 2 == 0:
            nc.vector.tensor_copy(et[:], ps_et[:])
        else:
            nc.scalar.copy(et[:], ps_et[:])
        ets[b] = et

    def emit_mm(b):
        et = ets.pop(b)
        ps_o = ps_big.tile([P, M], F32, space="PSUM", tag="ps_o",
                           name=f"ps_o{b}")
        nc.tensor.matmul(ps_o[:], lhsT=et[:], rhs=projT[:],
                         start=True, stop=True)
        o_sb = o_pool.tile([P, M], F32, tag="o_sb", name=f"o_sb{b}")
        if b % 2 == 0:
            nc.scalar.copy(o_sb[:], ps_o[:])
        else:
            nc.vector.tensor_copy(o_sb[:], ps_o[:])
        nc.sync.dma_start(out_r[:, :, b], o_sb[:])

    for b in range(NB):
        emit_trans(b)
        if b >= LAG:
            emit_mm(b - LAG)
    for b in range(NB - LAG, NB):
        emit_mm(b)


_CACHE: dict = {}


def _build(W: int):
    if W in _CACHE:
        return _CACHE[W]
    nc = bacc.Bacc("TRN2", target_bir_lowering=False, debug=False,
                   num_swdge_queues=N_QUEUES, dynamic_dma_scratch_size=65536)
    tok = nc.dram_tensor("token_ids", [S * W], I32, kind="ExternalInput").ap()
    table = nc.dram_tensor("embed_weight", [V, D], F32, kind="ExternalInput").ap()
    proj = nc.dram_tensor("proj_weight", [M, D], F32, kind="ExternalInput").ap()
    scale = nc.dram_tensor("scale", [1, 1], F32, kind="ExternalInput").ap()
    out = nc.dram_tensor("out", [S, M], F32, kind="ExternalOutput").ap()
    with tile.TileContext(nc) as tc:
        with ExitStack() as ctx:
            body(ctx, tc, out, tok, table, proj, scale, W)
    nc.compile()
    _CACHE[W] = nc
    return nc


def kernel(token_ids: np.ndarray, embed_weight: np.ndarray,
           proj_weight: np.ndarray, scale: np.ndarray) -> np.ndarray:
    token_ids = np.ascontiguousarray(token_ids)
    assert token_ids.shape == (B, S), token_ids.shape
    W = 2 if token_ids.dtype.itemsize == 8 else 1
    tok32 = token_ids.view(np.int32).reshape(B, S * W)
    table = np.ascontiguousarray(embed_weight, dtype=np.float32)
    proj = np.ascontiguousarray(proj_weight, dtype=np.float32)
    sc = np.asarray(scale, dtype=np.float32).reshape(1, 1)

    nc = _build(W)
    in_maps = [
        {
            "token_ids": np.ascontiguousarray(tok32[i]),
            "embed_weight": table,
            "proj_weight": proj,
            "scale": sc,
        }
        for i in range(B)
    ]
    res = run_bass_kernel_spmd(nc, in_maps, core_ids=list(range(B)))
    return np.stack([r["out"] for r in res.results], axis=0)


# revision 18
# speedup vs baseline: 1.1136x; 1.1009x over previous
"""Trainium2 Bass kernel: BigramHashEmbedding (hash -> embed gather -> proj -> scale).

Computation (per batch row, one NeuronCore per row, 8 rows total):
    h[0]  = 10239
    h[j]  = (36313*t[j] ^ 27191*t[j-1]) % 10239          (int32, j >= 1)
    e     = embed_weight[h]                               [S, 128] gather
    out   = (e @ proj_weight.T) * scale                   [S, 512]

Device strategy per core (S = 8192 tokens):
  * tokens are viewed int32 (lo-words of int64 if needed) and loaded into
    SBUF in [16, 512] layout (partition p holds tokens 512p..512p+511),
    replicated 8x across the 128 partitions (the dma_gather index tile must
    be "wrapped in 16 partitions and replicated across cores").
  * the bigram hash is computed on DVE/ACT with fp32-exact arithmetic:
    products are split (36313 = 141*256 + 217, 27191 = 106*256 + 55) so that
    every arithmetic op stays below 2^24 (the vector ALU is fp32 internally);
    >=2^24 values only ever pass through bitwise ops (shift/and/xor), which
    are bit-exact.  The mod-10239 is a limb decomposition
    X = u*2^21 + v*2^8 + w  ->  y = u*8396 + (v<<8) + w  (y < 2^24)
    followed by one fp32 reciprocal-multiply quotient and two +-m fixups.
  * one dma_gather per 2048 tokens fetches embedding rows straight from the
    DRAM table into SBUF slot layout [128, 16, 128] (row k%128 / block k//128).
    Gather slot k holds token 512*(k%16) + k//16.
  * per 128-token block: PE transpose (identity) -> PSUM -> SBUF, then
    PE matmul eT.T @ projT_scaled -> PSUM [128, 512] -> SBUF -> HWDGE DMA to
    the matching strided rows of the output.  Matmuls run as float32r
    (single-pass fp32, 1 cycle/row instead of 4).
  * proj [512, 128] is transposed on the PE at setup into projT [128, 512]
    and pre-scaled by `scale` (broadcast via a K=1 matmul with a ones row).
"""

from contextlib import ExitStack

import numpy as np

import concourse.bacc as bacc
import concourse.bass as bass
import concourse.mybir as mybir
import concourse.tile as tile
from concourse.bass_utils import run_bass_kernel_spmd
from concourse.masks import make_identity

AL = mybir.AluOpType
F32 = mybir.dt.float32
F32R = mybir.dt.float32r
I32 = mybir.dt.int32
I16 = mybir.dt.int16

B = 8           # batch rows == cores
S = 8192        # tokens per core
V = 10240       # hash table rows
D = 128         # embed dim
M = 512         # model dim
P = 128
MOD = 10239     # hash modulus (HASH_SIZE - 1)
SPT = S // 16   # tokens per index-partition = 512
NG = 8          # sub-gathers
TPG = S // NG   # tokens per gather = 2048
CPG = SPT // NG  # idx columns per gather group = 128
NB = S // P     # 128-token blocks = 64
BPG = NB // NG  # blocks per gather = 16
HASH_CHUNKS = (64, 64, 128, 256)   # progressive: short first chain, wide later
assert sum(HASH_CHUNKS) == SPT

# 36313 = 141*256 + 217 ; 27191 = 106*256 + 55
A_HI, A_LO = 141, 217
B_HI, B_LO = 106, 55
C21 = 8396      # 2^21 mod 10239
INV_M = 1.0 / MOD

USE_ACT_MUL = True   # run the big multiplies on the Scalar (ACT) engine
N_QUEUES = 4         # SWDGE queues for the gathers
USE_F32R = True      # float32r single-pass matmul (1 cyc/row instead of 4)
SIM_COMPAT = False   # add the >=MOD fixup (needed only under CoreSim's trunc convert)


def _mul(nc, out, in_, const):
    if USE_ACT_MUL:
        nc.scalar.mul(out, in_, float(const))
    else:
        nc.vector.tensor_scalar_mul(out, in_, float(const))


def _hash_chunk(nc, tmp, idx, toks_v, tm1, mask, offs, cs, n):
    """Emit ops computing idx[:, cs:cs+n] (int16 hash values).

    toks_v: [128, SPT, W] int32 view of the token tile (lo word at w=0).
    tm1:    [128, 1] int32, t[512p - 1] per partition (garbage at p%16==0).
    mask:   [128, 1] int32, (p % 16) != 0.
    offs:   [128, 1] int32, 10239 * (p % 16 == 0).
    """
    g = 0 if cs == 0 else 1  # only the cs==0 chunk handles the row head

    tcur = toks_v[:, cs:cs + n, 0:1]
    p1 = tmp.tile([P, n], I32, tag=f"p1_{n}")
    p2 = tmp.tile([P, n], I32, tag=f"p2_{n}")
    q1 = tmp.tile([P, n], I32, tag=f"q1_{n}")
    q2 = tmp.tile([P, n], I32, tag=f"q2_{n}")
    _mul(nc, p1[:], tcur, A_LO)
    _mul(nc, p2[:], tcur, A_HI)
    if g == 0:
        tprev = toks_v[:, 0:n - 1, 0:1]
        _mul(nc, q1[:, 1:n], tprev, B_LO)
        _mul(nc, q2[:, 1:n], tprev, B_HI)
        _mul(nc, q1[:, 0:1], tm1[:], B_LO)
        _mul(nc, q2[:, 0:1], tm1[:], B_HI)
    else:
        tprev = toks_v[:, cs - 1:cs + n - 1, 0:1]
        _mul(nc, q1[:], tprev, B_LO)
        _mul(nc, q2[:], tprev, B_HI)

    # A>>8 = p2 + (p1>>8);  B>>8 = q2 + (q1>>8)   (both < 2^23, exact)
    # (the compiler rejects bitwise op0 fused with arith op1, so shift and
    # add are separate instructions)
    ah = tmp.tile([P, n], I32, tag=f"ah_{n}")
    bh = tmp.tile([P, n], I32, tag=f"bh_{n}")
    t1 = tmp.tile([P, n], I32, tag=f"t1_{n}")
    nc.vector.tensor_single_scalar(t1[:], p1[:], 8, op=AL.logical_shift_right)
    nc.vector.tensor_add(ah[:], t1[:], p2[:])
    nc.vector.tensor_single_scalar(t1[:], q1[:], 8, op=AL.logical_shift_right)
    nc.vector.tensor_add(bh[:], t1[:], q2[:])
    # X>>8 and X low byte (in low 8 bits of xl)
    xh = tmp.tile([P, n], I32, tag=f"xh_{n}")
    xl = tmp.tile([P, n], I32, tag=f"xl_{n}")
    nc.vector.tensor_tensor(xh[:], ah[:], bh[:], op=AL.bitwise_xor)
    nc.vector.tensor_tensor(xl[:], p1[:], q1[:], op=AL.bitwise_xor)

    # y = (xh>>13)*8396 + ((xh & 8191) << 8) + (xl & 255)   ( < 2^24 )
    w1 = tmp.tile([P, n], I32, tag=f"w1_{n}")
    w2 = tmp.tile([P, n], I32, tag=f"w2_{n}")
    nc.vector.tensor_single_scalar(w1[:], xh[:], 13, op=AL.logical_shift_right)
    nc.vector.tensor_scalar_mul(w1[:], w1[:], float(C21))
    nc.vector.tensor_scalar(w2[:], xh[:], 8191, 8,
                            op0=AL.bitwise_and, op1=AL.logical_shift_left)
    w3 = tmp.tile([P, n], I32, tag=f"w3_{n}")
    nc.vector.tensor_add(w3[:], w1[:], w2[:])
    y = tmp.tile([P, n], I32, tag=f"y_{n}")
    nc.vector.tensor_single_scalar(y[:], xl[:], 255, op=AL.bitwise_and)
    nc.vector.tensor_add(y[:], y[:], w3[:])

    # r = y - trunc(y/m)*m, then two +-m fixups
    qt = tmp.tile([P, n], I32, tag=f"qt_{n}")
    _mul(nc, qt[:], y[:], INV_M)
    r = tmp.tile([P, n], I32, tag=f"r_{n}")
    nc.vector.scalar_tensor_tensor(r[:], qt[:], -float(MOD), y[:],
                                   op0=AL.mult, op1=AL.add)
    if SIM_COMPAT:
        f1 = tmp.tile([P, n], I32, tag=f"f1_{n}")
        nc.vector.tensor_single_scalar(f1[:], r[:], float(MOD), op=AL.is_ge)
        nc.vector.scalar_tensor_tensor(r[:], f1[:], -float(MOD), r[:],
                                       op0=AL.mult, op1=AL.add)
    f2 = tmp.tile([P, n], I32, tag=f"f2_{n}")
    nc.vector.tensor_single_scalar(f2[:], r[:], 0.0, op=AL.is_lt)
    nc.vector.scalar_tensor_tensor(r[:], f2[:], float(MOD), r[:],
                                   op0=AL.mult, op1=AL.add)

    if g == 0:
        # token 0 (partition p%16==0, col 0): h = MOD
        nc.vector.tensor_mul(r[:, 0:1], r[:, 0:1], mask[:])
        nc.vector.tensor_add(r[:, 0:1], r[:, 0:1], offs[:])

    nc.vector.tensor_copy(idx[:, cs:cs + n], r[:])


def body(ctx: ExitStack, tc: tile.TileContext, out_ap, tok_ap, table_ap,
         proj_ap, scale_ap, W: int):
    """Emit the per-core kernel. tok_ap is int32 [S*W] (W=2 -> int64 lo/hi)."""
    nc = tc.nc

    const = ctx.enter_context(tc.tile_pool(name="const", bufs=1))
    tmp = ctx.enter_context(tc.tile_pool(name="tmp", bufs=2))
    gpool = ctx.enter_context(tc.tile_pool(name="gpool", bufs=1))
    et_pool = ctx.enter_context(tc.tile_pool(name="et", bufs=6))
    o_pool = ctx.enter_context(tc.tile_pool(name="osb", bufs=8))

    # ---- tokens first (they gate the hash -> gather critical path) ----
    FW = SPT * W
    tokv = tok_ap.rearrange("(p f) -> p f", p=16)
    toks = const.tile([P, FW], I32)
    tm1 = const.tile([P, W], I32)
    nc.gpsimd.memset(tm1[:], 0)
    nc.sync.dma_start(toks[:], tokv[None].broadcast_to([8, 16, FW]))
    for r in range(8):
        # t[512q - 1] for q>=1: last element of the previous partition
        nc.sync.dma_start(tm1[16 * r + 1:16 * (r + 1), :],
                          tokv[0:15, FW - W:FW])
    toks_v = toks.rearrange("p (s w) -> p s w", w=W)

    # partition masks for the token-0 override
    pi = const.tile([P, 1], I32)
    nc.gpsimd.iota(pi[:], pattern=[[0, 1]], base=0, channel_multiplier=1)
    mask = const.tile([P, 1], I32)
    nc.vector.tensor_single_scalar(mask[:], pi[:], 15, op=AL.bitwise_and)
    nc.vector.tensor_single_scalar(mask[:], mask[:], 0.0, op=AL.not_equal)
    offs = const.tile([P, 1], I32)
    nc.vector.tensor_scalar(offs[:], mask[:], -float(MOD), float(MOD),
                            op0=AL.mult, op1=AL.add)

    idx = const.tile([P, SPT], I16)
    g_sb = gpool.tile([P, NB, P], F32R if USE_F32R else F32)

    # hash + gathers (each chunk covers whole gathers; gather = CPG columns)
    cs = 0
    for n in HASH_CHUNKS:
        _hash_chunk(nc, tmp, idx, toks_v, tm1[:, 0:1], mask, offs, cs, n)
        for g in range(cs // CPG, (cs + n) // CPG):
            nc.gpsimd.dma_gather(
                g_sb[:, BPG * g:BPG * (g + 1), :],
                table_ap.bitcast(F32R) if USE_F32R else table_ap,
                idx[:, CPG * g:CPG * (g + 1)],
                num_idxs=TPG,
                num_idxs_reg=TPG,
                elem_size=D,
                single_packet=False,
                queue_num=g % N_QUEUES,
            )
        cs += n

    # ---- setup: identity, projT (transposed + pre-scaled) ----
    ps_setup = tc.alloc_tile_pool(name="ps_setup", bufs=1, space="PSUM")
    ident_f = const.tile([P, P], F32)
    make_identity(nc, ident_f[:])
    if USE_F32R:
        # f32r consumers need f32r producers; DVE copy does the rounding
        ident = const.tile([P, P], F32R)
        nc.vector.tensor_copy(ident[:], ident_f[:])
    else:
        ident = ident_f

    # scale broadcast [1,1] -> [128,1] via K=1 matmul with a ones row
    sc_in = const.tile([1, 1], F32)
    nc.sync.dma_start(sc_in[:], scale_ap)
    ones = const.tile([1, P], F32)
    nc.gpsimd.memset(ones[:], 1.0)
    ps_sc = ps_setup.tile([P, 1], F32, space="PSUM", tag="ps_sc")
    nc.tensor.matmul(ps_sc[:], lhsT=ones[:], rhs=sc_in[:], start=True, stop=True)
    sc_b = const.tile([P, 1], F32)
    nc.vector.tensor_copy(sc_b[:], ps_sc[:])

    projT = const.tile([P, M], F32R if USE_F32R else F32)
    for c in range(M // P):
        pch = tmp.tile([P, P], F32, tag="pch")
        nc.sync.dma_start(pch[:], proj_ap[c * P:(c + 1) * P, :])
        ps_t = ps_setup.tile([P, P], F32, space="PSUM", tag="ps_t")
        nc.tensor.transpose(ps_t[:], pch[:], ident_f[:])
        nc.vector.tensor_copy(projT[:, c * P:(c + 1) * P], ps_t[:])
    nc.vector.tensor_scalar_mul(projT[:], projT[:], sc_b[:, 0:1])
    ps_setup.release()
    ps_small = ctx.enter_context(tc.tile_pool(name="ps_small", bufs=4, space="PSUM"))
    ps_big = ctx.enter_context(tc.tile_pool(name="ps_big", bufs=4, space="PSUM"))

    # DRAM-side dims iterate (r outer, q inner, m) so the flat order pairs
    # SBUF partition p = r*16 + q with output row 512q + 8s + r.
    out_r = out_ap.rearrange("(q s r) m -> r q s m", q=16, s=NB, r=8)

    # Software-pipelined block loop: the PE issues transpose(b+LAG) before
    # matmul(b) so the PSUM->SBUF copy latency of eT stays off the PE's
    # in-order critical path.
    LAG = 2
    ets = {}

    def emit_trans(b):
        ps_et = ps_small.tile([P, P], F32R if USE_F32R else F32, space="PSUM",
                              tag="ps_et", name=f"ps_et{b}")
        nc.tensor.transpose(ps_et[:], g_sb[:, b, :], ident[:])
        et = et_pool.tile([P, P], F32R if USE_F32R else F32, tag="et",
                          name=f"et{b}")
        nc.vector.tensor_copy(et[:], ps_et[:])
        ets[b] = et

    def emit_mm(b):
        et = ets.pop(b)
        ps_o = ps_big.tile([P, M], F32, space="PSUM", tag="ps_o",
                           name=f"ps_o{b}")
        nc.tensor.matmul(ps_o[:], lhsT=et[:], rhs=projT[:],
                         start=True, stop=True)
        o_sb = o_pool.tile([P, M], F32, tag="o_sb", name=f"o_sb{b}")
        nc.scalar.copy(o_sb[:], ps_o[:])
        nc.sync.dma_start(out_r[:, :, b], o_sb[:])

    for b in range(NB):
        emit_trans(b)
        if b >= LAG:
            emit_mm(b - LAG)
    for b in range(NB - LAG, NB):
        emit_mm(b)


_CACHE: dict = {}


def _build(W: int):
    if W in _CACHE:
        return _CACHE[W]
    nc = bacc.Bacc("TRN2", target_bir_lowering=False, debug=False,
                   num_swdge_queues=N_QUEUES, dynamic_dma_scratch_size=65536)
    tok = nc.dram_tensor("token_ids", [S * W], I32, kind="ExternalInput").ap()
    table = nc.dram_tensor("embed_weight", [V, D], F32, kind="ExternalInput").ap()
    proj = nc.dram_tensor("proj_weight", [M, D], F32, kind="ExternalInput").ap()
    scale = nc.dram_tensor("scale", [1, 1], F32, kind="ExternalInput").ap()
    out = nc.dram_tensor("out", [S, M], F32, kind="ExternalOutput").ap()
    with tile.TileContext(nc) as tc:
        with ExitStack() as ctx:
            body(ctx, tc, out, tok, table, proj, scale, W)
    nc.compile()
    _CACHE[W] = nc
    return nc


def kernel(token_ids: np.ndarray, embed_weight: np.ndarray,
           proj_weight: np.ndarray, scale: np.ndarray) -> np.ndarray:
    token_ids = np.ascontiguousarray(token_ids)
    assert token_ids.shape == (B, S), token_ids.shape
    W = 2 if token_ids.dtype.itemsize == 8 else 1
    tok32 = token_ids.view(np.int32).reshape(B, S * W)
    table = np.ascontiguousarray(embed_weight, dtype=np.float32)
    proj = np.ascontiguousarray(proj_weight, dtype=np.float32)
    sc = np.asarray(scale, dtype=np.float32).reshape(1, 1)

    nc = _build(W)
    in_maps = [
        {
            "token_ids": np.ascontiguousarray(tok32[i]),
            "embed_weight": table,
            "proj_weight": proj,
            "scale": sc,
        }
        for i in range(B)
    ]
    res = run_bass_kernel_spmd(nc, in_maps, core_ids=list(range(B)))
    return np.stack([r["out"] for r in res.results], axis=0)
